# revision 3
# baseline (speedup 1.0000x reference)
"""Trainium2 Bass kernel for nn_Encoder_RT (5-stage streaming conv encoder).

Strategy: pure data parallel over 8 NeuronCores (batch 8192 -> 1024/core).
Per core, batches are processed in tiles of Bt=64. All five conv stages are
fused on-chip: each stage's input tile holds the DMA-loaded cache frame in
partitions [0,C) and the previous stage's output (written by the ScalarE
Prelu evacuation) in partitions [C,2C). Convs run on TensorE with channels
(x time) on the contraction dim and (batch, freq) on the moving free dim;
BatchNorm is applied via the activation's per-partition scale/bias and PReLU
via the native Prelu activation function with per-partition alpha.

Stage 1 (K would be only 4) uses a x5-replicated input tile (one partition
group per kf tap, DMA-replicated from HBM) so the whole 5-tap conv is a
single K=20 matmul per psum chunk.

Outputs c1..c4 are byte-identical to x1..x4 and are returned as aliases of
the same arrays; c0 is the host-side transpose of x (pure data movement).
"""

import numpy as np

import concourse.bacc as bacc
import concourse.mybir as mybir
import concourse.tile as tile
from concourse import bass_utils

F32 = mybir.dt.float32

B_FULL = 8192
N_CORES = 8
B_CORE = B_FULL // N_CORES  # 1024


def build_nc(b_core=B_CORE, bt=64):
    """Build the per-core Bass module (SPMD: same program on every core)."""
    n_tiles = b_core // bt
    nc = bacc.Bacc("TRN2", target_bir_lowering=False, debug=False)

    # --- DRAM I/O (per-core shard shapes, squeezed to 3D) ---
    xt_d = nc.dram_tensor("xt", (b_core, 2, 256), F32, kind="ExternalInput")
    c0_d = nc.dram_tensor("cin0", (b_core, 2, 256), F32, kind="ExternalInput")
    c1_d = nc.dram_tensor("cin1", (b_core, 32, 127), F32, kind="ExternalInput")
    c2_d = nc.dram_tensor("cin2", (b_core, 32, 64), F32, kind="ExternalInput")
    c3_d = nc.dram_tensor("cin3", (b_core, 32, 64), F32, kind="ExternalInput")
    c4_d = nc.dram_tensor("cin4", (b_core, 64, 64), F32, kind="ExternalInput")
    w1_d = nc.dram_tensor("w1", (20, 32), F32, kind="ExternalInput")
    w2_d = nc.dram_tensor("w2", (64, 3, 32), F32, kind="ExternalInput")
    w3_d = nc.dram_tensor("w3", (64, 3, 32), F32, kind="ExternalInput")
    w4_d = nc.dram_tensor("w4", (64, 3, 64), F32, kind="ExternalInput")
    w5_d = nc.dram_tensor("w5", (128, 3, 128), F32, kind="ExternalInput")
    sba_d = nc.dram_tensor("sba", (128, 15), F32, kind="ExternalInput")
    x1_d = nc.dram_tensor("x1", (b_core, 32, 127), F32, kind="ExternalOutput")
    x2_d = nc.dram_tensor("x2", (b_core, 32, 64), F32, kind="ExternalOutput")
    x3_d = nc.dram_tensor("x3", (b_core, 32, 64), F32, kind="ExternalOutput")
    x4_d = nc.dram_tensor("x4", (b_core, 64, 64), F32, kind="ExternalOutput")
    x5_d = nc.dram_tensor("x5", (b_core, 128, 64), F32, kind="ExternalOutput")

    PRELU = mybir.ActivationFunctionType.Prelu

    with tile.TileContext(nc) as tc:
        with (
            tc.tile_pool(name="sb", bufs=1) as pool,
            tc.tile_pool(name="ps", bufs=4, space="PSUM") as pp,
        ):
            # --- persistent tiles ---
            # T1 row = 4*kf + 2*t + c ; T1[row, b, j] = in_pad[c, t, j + kf - 1]
            t1 = pool.tile([20, bt, 254], F32)
            t2 = pool.tile([64, bt, 129], F32)  # rows 0-31 cache1 / 32-63 x1
            t3 = pool.tile([64, bt, 66], F32)   # rows 0-31 cache2 / 32-63 x2
            t4 = pool.tile([64, bt, 66], F32)   # rows 0-31 cache3 / 32-63 x3
            t5 = pool.tile([128, bt, 66], F32)  # rows 0-63 cache4 / 64-127 x4
            t6 = pool.tile([128, bt, 64], F32)  # x5
            tw1 = pool.tile([20, 32], F32)
            tw2 = pool.tile([64, 3, 32], F32)
            tw3 = pool.tile([64, 3, 32], F32)
            tw4 = pool.tile([64, 3, 64], F32)
            tw5 = pool.tile([128, 3, 128], F32)
            sba = pool.tile([128, 15], F32)

            nc.sync.dma_start(tw1[:], w1_d[:])
            nc.sync.dma_start(tw2[:], w2_d[:])
            nc.sync.dma_start(tw3[:], w3_d[:])
            nc.sync.dma_start(tw4[:], w4_d[:])
            nc.sync.dma_start(tw5[:], w5_d[:])
            nc.sync.dma_start(sba[:], sba_d[:])

            # zero-pad columns (written once; per-iter DMAs never touch them)
            nc.gpsimd.memset(t1[0:4, :, 0:1], 0.0)
            nc.gpsimd.memset(t2[:, :, 0:1], 0.0)
            nc.gpsimd.memset(t2[:, :, 128:129], 0.0)
            for t in (t3, t4, t5):
                nc.gpsimd.memset(t[:, :, 0:1], 0.0)
                nc.gpsimd.memset(t[:, :, 65:66], 0.0)

            def act(dst, src, s, r0, r1):  # fused BN + PReLU, stage s (0-based)
                nc.scalar.activation(
                    dst, src, PRELU,
                    bias=sba[r0:r1, 3 * s + 1:3 * s + 2],
                    scale=sba[r0:r1, 3 * s + 0:3 * s + 1],
                    alpha=sba[r0:r1, 3 * s + 2:3 * s + 3],
                )

            for it in range(n_tiles):
                b0 = it * bt

                def dsl(d, f0, f1):  # dram slice -> (c, b, f) view
                    return d[b0:b0 + bt, :, f0:f1].rearrange("b c f -> c b f")

                # --- loads ---
                # stage-1 replicated tile: kf group r holds in_pad[c,t,j+r-1]
                for r in range(5):
                    sf0, sf1 = max(0, r - 1), min(256, 253 + r)
                    j0 = max(0, 1 - r)
                    jn = sf1 - sf0
                    nc.sync.dma_start(
                        t1[4 * r + 0:4 * r + 2, :, j0:j0 + jn], dsl(c0_d, sf0, sf1))
                    nc.sync.dma_start(
                        t1[4 * r + 2:4 * r + 4, :, j0:j0 + jn], dsl(xt_d, sf0, sf1))
                nc.sync.dma_start(t2[0:32, :, 1:128], dsl(c1_d, 0, 127))
                nc.sync.dma_start(t3[0:32, :, 1:65], dsl(c2_d, 0, 64))
                nc.sync.dma_start(t4[0:32, :, 1:65], dsl(c3_d, 0, 64))
                nc.sync.dma_start(t5[0:64, :, 1:65], dsl(c4_d, 0, 64))

                # --- stage 1: K=20 single matmul per 4-batch chunk ---
                for g in range(bt // 4):
                    b = 4 * g
                    ps = pp.tile([128, 4, 127], F32, tag="ps")
                    nc.tensor.matmul(
                        ps[32:64], tw1[:], t1[:, b:b + 4, 0:253:2],
                        start=True, stop=True)
                    act(t2[32:64, b:b + 4, 1:128], ps[32:64], 0, 32, 64)

                # --- stages 2-5: 3 accumulating kf matmuls per 8-batch chunk ---
                for g in range(bt // 8):
                    b = 8 * g
                    ps = pp.tile([128, 8, 64], F32, tag="ps")
                    for kf in range(3):
                        nc.tensor.matmul(
                            ps[32:64], tw2[:, kf, :],
                            t2[:, b:b + 8, kf:kf + 127:2],
                            start=(kf == 0), stop=(kf == 2))
                    act(t3[32:64, b:b + 8, 1:65], ps[32:64], 1, 32, 64)
                for g in range(bt // 8):
                    b = 8 * g
                    ps = pp.tile([128, 8, 64], F32, tag="ps")
                    for kf in range(3):
                        nc.tensor.matmul(
                            ps[32:64], tw3[:, kf, :], t3[:, b:b + 8, kf:kf + 64],
                            start=(kf == 0), stop=(kf == 2))
                    act(t4[32:64, b:b + 8, 1:65], ps[32:64], 2, 32, 64)
                for g in range(bt // 8):
                    b = 8 * g
                    ps = pp.tile([128, 8, 64], F32, tag="ps")
                    for kf in range(3):
                        nc.tensor.matmul(
                            ps[64:128], tw4[:, kf, :], t4[:, b:b + 8, kf:kf + 64],
                            start=(kf == 0), stop=(kf == 2))
                    act(t5[64:128, b:b + 8, 1:65], ps[64:128], 3, 64, 128)
                for g in range(bt // 8):
                    b = 8 * g
                    ps = pp.tile([128, 8, 64], F32, tag="ps")
                    for kf in range(3):
                        nc.tensor.matmul(
                            ps[0:128], tw5[:, kf, :], t5[:, b:b + 8, kf:kf + 64],
                            start=(kf == 0), stop=(kf == 2))
                    act(t6[:, b:b + 8, :], ps[0:128], 4, 0, 128)

                # --- stores ---
                def dso(d):
                    return d[b0:b0 + bt, :, :].rearrange("b c f -> c b f")

                nc.sync.dma_start(dso(x1_d), t2[32:64, :, 1:128])
                nc.sync.dma_start(dso(x2_d), t3[32:64, :, 1:65])
                nc.sync.dma_start(dso(x3_d), t4[32:64, :, 1:65])
                nc.sync.dma_start(dso(x4_d), t5[64:128, :, 1:65])
                nc.sync.dma_start(dso(x5_d), t6[:])

    nc.compile()
    return nc


def prep_params(params):
    """Fold BN into per-channel scale/bias; pack weights as lhsT tiles."""
    eps = 1e-8
    out = {}
    sba = np.zeros((128, 15), np.float32)
    rows = [(32, 64), (32, 64), (32, 64), (64, 128), (0, 128)]
    for i, key in enumerate(["s1", "s2", "s3", "s4", "s5"]):
        p = params[key]
        W = np.asarray(p["W"], np.float32)     # [O, C, 2, kf]
        s = np.asarray(p["gamma"], np.float32) / np.sqrt(
            np.asarray(p["var"], np.float32) + eps)
        bias = (np.asarray(p["b"], np.float32)
                - np.asarray(p["mean"], np.float32)) * s + np.asarray(
                    p["beta"], np.float32)
        r0, r1 = rows[i]
        sba[r0:r1, 3 * i + 0] = s
        sba[r0:r1, 3 * i + 1] = bias
        sba[r0:r1, 3 * i + 2] = np.asarray(p["alpha"], np.float32)
        O, C, T, KF = W.shape
        if i == 0:
            w1 = np.zeros((20, 32), np.float32)
            for kf in range(5):
                for t in range(2):
                    for c in range(2):
                        w1[4 * kf + 2 * t + c, :] = W[:, c, t, kf]
            out["w1"] = w1
        else:
            # [O,C,T,KF] -> [T*C, KF, O] with p = t*C + c
            out[f"w{i + 1}"] = np.ascontiguousarray(
                W.transpose(2, 1, 3, 0).reshape(T * C, KF, O))
    out["sba"] = sba
    return out


_NC_CACHE = {}


def _get_nc(b_core=B_CORE, bt=64):
    key = (b_core, bt)
    if key not in _NC_CACHE:
        _NC_CACHE[key] = build_nc(b_core, bt)
    return _NC_CACHE[key]


def run(nc, inputs, b_core, n_cores, trace=False):
    """Shard inputs, run SPMD on n_cores, gather outputs."""
    x = np.asarray(inputs["x"], np.float32)
    xt = np.ascontiguousarray(x.transpose(0, 3, 2, 1))  # [B,2,1,256] == c0
    w = prep_params(inputs["params"])
    sh = {
        "xt": xt.reshape(-1, 2, 256),
        "cin0": np.asarray(inputs["cache"], np.float32).reshape(-1, 2, 256),
        "cin1": np.asarray(inputs["cache1"], np.float32).reshape(-1, 32, 127),
        "cin2": np.asarray(inputs["cache2"], np.float32).reshape(-1, 32, 64),
        "cin3": np.asarray(inputs["cache3"], np.float32).reshape(-1, 32, 64),
        "cin4": np.asarray(inputs["cache4"], np.float32).reshape(-1, 64, 64),
    }
    in_maps = []
    for c in range(n_cores):
        m = {k: np.ascontiguousarray(v[c * b_core:(c + 1) * b_core])
             for k, v in sh.items()}
        m.update(w)
        in_maps.append(m)
    res = bass_utils.run_bass_kernel_spmd(
        nc, in_maps, core_ids=list(range(n_cores)), trace=trace)
    B = b_core * n_cores

    def gather(name, cshape):
        return np.concatenate(
            [res.results[c][name] for c in range(n_cores)], axis=0
        ).reshape(B, cshape[0], 1, cshape[1])

    x1 = gather("x1", (32, 127))
    x2 = gather("x2", (32, 64))
    x3 = gather("x3", (32, 64))
    x4 = gather("x4", (64, 64))
    x5 = gather("x5", (128, 64))
    c0 = xt[: B]
    return (x1, x2, x3, x4, x5, c0, x1, x2, x3, x4), res


def kernel(x, cache, cache1, cache2, cache3, cache4, params):
    nc = _get_nc()
    outs, _ = run(
        nc,
        dict(x=x, cache=cache, cache1=cache1, cache2=cache2, cache3=cache3,
             cache4=cache4, params=params),
        B_CORE, N_CORES)
    return outs
